# revision 9
# baseline (speedup 1.0000x reference)
"""GAT (2-layer, 4/1 heads) on 8 trn2 NeuronCores via Bass/Tile.

Strategy (v2, dst-partitioned edge-major with host-built transposed one-hots):
- Node permutation: nodes dealt degree-sorted (snake) into 8*NT windows of 128
  dsts so every window has near-equal edge count (minimal tile padding).
- Phase A (replicated): every core computes the FULL rec table
  [h | s | d] = xT-tile @ [W1 | W1@a_src | W1@a_dst] for all nodes (no
  collective), writing 512B-stride rows + a per-window transposed d table.
- GAT layer: edges (+self loops) grouped per (window, src-quarter), 128/tile.
  Per tile: per-edge d via PE matmul with a host-uploaded transposed one-hot
  (OT); bulk exp(lrelu(s+d))+mask per batch; bulk per-head weighting of the
  gathered records; aggregation via (DVE-built one-hot) @ weighted-rec matmul
  accumulating [dst,132] in PSUM; per-window softmax-normalize + bias + ELU.
- Records fetched with InstDMAGatherAnt (int16 idx, quarter-relative).
- Layer-1 output stored transposed [128f, NPC]; one AllGather; Phase C
  (replicated) computes the layer-2 rec table for all nodes.
- Final: graph mean-pool via one-hot matmuls, AllReduce, tiny linear.
"""

import math

import numpy as np
import ml_dtypes

import concourse.bass as bass
import concourse.mybir as mybir
import concourse.tile as tile
from concourse import bacc
from concourse.bass_utils import run_bass_kernel_spmd
from concourse.masks import make_identity

NCORES = 8
P = 128
NEG_SLOPE = 0.2
BWIN = 3          # windows per batch
CH = 8            # tiles per gather chunk (1024-index HW limit)

f16 = mybir.dt.float16
f32 = mybir.dt.float32
i16 = mybir.dt.int16

_last_exec_ns = None


def _install_ntff_hook():
    """Provide antenv.axon_hooks (missing on this image) so trace=True works."""
    import sys
    import types
    try:
        from antenv import axon_hooks  # noqa: F401
        return
    except ImportError:
        pass
    import antenv
    mod = types.ModuleType("antenv.axon_hooks")
    mod._hook = None
    mod.set_axon_ntff_profile_hook = lambda h: setattr(mod, "_hook", h)
    mod.get_axon_ntff_profile_hook = lambda: mod._hook
    sys.modules["antenv.axon_hooks"] = mod
    antenv.axon_hooks = mod
    try:
        from trn_agent_boot.trn_boot import _ntff_profile_via_ctypes
        mod._hook = _ntff_profile_via_ctypes("/opt/axon/libaxon_pjrt.so")
    except Exception:
        mod._hook = None
    import concourse.bass_utils as bu
    bu.upload_artifacts = lambda tmpdir: f"local:{tmpdir}"


# ---------------------------------------------------------------- host helpers

def _wrap16(flat, pad_val=0):
    """int16 index list -> [128, ceil(n/16)] wrapped+replicated layout."""
    n = len(flat)
    cols = (n + 15) // 16
    a = np.full(cols * 16, pad_val, np.int16)
    a[:n] = flat
    w = a.reshape(cols, 16).T  # [16, cols]
    return np.tile(w, (8, 1))  # [128, cols]


def _slotmajor(flat, T, dtype):
    """slot-stream [T*128] -> [128, T] (slot i -> partition i%128, tile i//128)."""
    return np.ascontiguousarray(flat.reshape(T, P).T.astype(dtype))


class Meta:
    pass


def _host_prep(x, edge_index, batch, heads, hid):
    N = x.shape[0]
    NT = (N + NCORES * P - 1) // (NCORES * P)   # windows per core
    GW = NCORES * NT                             # global windows
    Npad = GW * P
    assert GW % 4 == 0
    QN = Npad // 4
    assert QN < 32768

    E0 = edge_index.shape[1]
    src = np.concatenate([np.asarray(edge_index[0]), np.arange(N)]).astype(np.int64)
    dst = np.concatenate([np.asarray(edge_index[1]), np.arange(N)]).astype(np.int64)

    # degree-balanced snake deal of nodes into GW windows of 128 slots
    deg = np.bincount(dst, minlength=N)
    order = np.argsort(-deg, kind="stable")
    node2slot = np.empty(N, np.int64)
    idx = np.arange(N)
    r = idx // GW                                 # round = slot-within-window
    pos = idx % GW
    W = np.where(r % 2 == 0, pos, GW - 1 - pos)   # snake
    node2slot[order] = W * P + r
    # global window W -> (core W%8, local window W//8)

    sslot = node2slot[src]
    dslot = node2slot[dst]
    Wd = dslot // P
    core_of = Wd % NCORES
    w_of = Wd // NCORES
    t_of = dslot % P
    q_of = sslot // QN

    # per (core, local window, quarter) edge lists
    cell = [[[None] * 4 for _ in range(NT)] for _ in range(NCORES)]
    for c in range(NCORES):
        mc = core_of == c
        sc, wc, tc, qc = sslot[mc], w_of[mc], t_of[mc], q_of[mc]
        for w in range(NT):
            mw = wc == w
            sw, tw, qw = sc[mw], tc[mw], qc[mw]
            for q in range(4):
                mq = qw == q
                cell[c][w][q] = (sw[mq], tw[mq])

    # equalized tile counts per (window, quarter)
    Twq = np.zeros((NT, 4), np.int64)
    for w in range(NT):
        for q in range(4):
            mx = max(len(cell[c][w][q][0]) for c in range(NCORES))
            Twq[w, q] = (mx + P - 1) // P

    m = Meta()
    m.N, m.NT, m.GW, m.Npad, m.QN = N, NT, GW, Npad, QN
    m.NPC_pad = NT * P
    m.heads, m.hid = heads, hid
    m.Twq = Twq
    m.node2slot = node2slot

    NB = (NT + BWIN - 1) // BWIN
    m.NB = NB
    m.batches = []
    for b in range(NB):
        ws = list(range(b * BWIN, min((b + 1) * BWIN, NT)))
        Rq = [int(Twq[ws, q].sum()) for q in range(4)]
        Tb = sum(Rq)
        reg_base = np.cumsum([0] + Rq)[:4]
        blk = {}
        for q in range(4):
            off = reg_base[q]
            for w in ws:
                blk[(w, q)] = int(off)
                off += int(Twq[w, q])
        m.batches.append(dict(ws=ws, Rq=Rq, Tb=Tb, blk=blk, reg_base=reg_base))

    m.rec_cols = []   # idx col counts per (b,q)
    m.li_cols = []    # li/mask col counts per b (=Tb)
    TbSum = sum(B["Tb"] for B in m.batches)
    m.TbSum = TbSum

    I128 = np.eye(P, dtype=np.float16)
    per_core = []
    for c in range(NCORES):
        rec_idx_cols = []
        li_all = np.zeros(TbSum * P, np.int64)
        mask_all = np.zeros(TbSum * P, np.float32)
        pos0 = 0
        for b in range(NB):
            B = m.batches[b]
            for q in range(4):
                r_flat = np.zeros(B["Rq"][q] * P, np.int64)
                for w in B["ws"]:
                    sw, tw = cell[c][w][q]
                    t0 = B["blk"][(w, q)] - B["reg_base"][q]
                    nsl = int(Twq[w, q]) * P
                    k = len(sw)
                    rr = np.zeros(nsl, np.int64)
                    ll = np.zeros(nsl, np.int64)
                    mm_ = np.full(nsl, -30000.0, np.float32)
                    rr[:k] = sw - q * QN
                    ll[:k] = tw
                    mm_[:k] = 0.0
                    r_flat[t0 * P:t0 * P + nsl] = rr
                    g0 = pos0 + (B["reg_base"][q] + t0) * P
                    li_all[g0:g0 + nsl] = ll
                    mask_all[g0:g0 + nsl] = mm_
                rec_idx_cols.append(_wrap16(r_flat.astype(np.int16)))
                if c == 0:
                    m.rec_cols.append(rec_idx_cols[-1].shape[1])
            if c == 0:
                m.li_cols.append(B["Tb"])
            pos0 += B["Tb"] * P
        li_sm = _slotmajor(li_all, TbSum, np.float16)
        mask_sm = _slotmajor(mask_all, TbSum, np.float32)
        OT = np.ascontiguousarray(
            I128[:, li_all.astype(np.int64).reshape(TbSum, P)]
            .transpose(0, 1, 2).reshape(P, TbSum * P))
        # OT[t, j*128+s] = 1 iff li(slot s of tile j) == t ; li_all is
        # slot-stream (tile-major), so cols are already (j, s)-ordered.
        dwidx = np.zeros(P, np.int64)
        dwidx[:NT] = np.arange(NT) * NCORES + c    # global W of local w
        per_core.append(dict(
            rec_idx=np.concatenate(rec_idx_cols, 1),
            li=li_sm, mask=mask_sm, OT=OT.astype(np.float16),
            dwidx=_wrap16(dwidx.astype(np.int16)),
        ))

    # graph pooling metadata (slot order)
    batch = np.asarray(batch).astype(np.int64)
    G = int(np.max(batch)) + 1
    m.G = G
    assert G <= 256
    counts = np.bincount(batch, minlength=256)
    m.recip = (1.0 / np.maximum(counts, 1)).astype(np.float32)
    for c in range(NCORES):
        gid = np.full(m.NPC_pad, -1, np.int64)
        # local row w*128+t <-> global slot (w*8+c)*128+t
        slots = ((np.arange(m.NPC_pad) // P) * NCORES + c) * P + (np.arange(m.NPC_pad) % P)
        node_of_slot = np.full(Npad, -1, np.int64)
        node_of_slot[node2slot] = np.arange(N)
        nds = node_of_slot[slots]
        real = nds >= 0
        gid[real] = batch[nds[real]]
        gA = gid.astype(np.float64)
        gB = np.where(gid >= 0, gid - 128, -1).astype(np.float64)
        per_core[c]["gidA"] = _slotmajor(gA, NT, np.float16)
        per_core[c]["gidB"] = _slotmajor(gB, NT, np.float16)
    m.per_core = per_core
    return m


# ---------------------------------------------------------------- raw dma_gather

def _dma_gather_raw(gp, out_ap, in_ap, idxs_ap, num_idxs, elem_size, elem_step,
                    queue_num=0):
    """dma_gather without the elem%256B assert (stride must be 256B-mult)."""
    from concourse import ap_utils
    from concourse._compat import exact_div
    assert idxs_ap.dtype == i16
    assert in_ap.dtype == out_ap.dtype
    assert ap_utils.ap_is_contiguous(in_ap.ap[1:])
    assert ap_utils.ap_is_contiguous(out_ap.ap[1:])
    assert ap_utils.ap_is_contiguous(idxs_ap.ap[1:])
    assert in_ap.ap[0][0] == elem_step
    stride_bytes = elem_step * mybir.dt.size(in_ap.dtype)
    stride_256 = exact_div(stride_bytes, 256)
    assert stride_256 < 256
    _in_ap = gp.lower_ap_dma(in_ap, for_custom_bir_dma=True)
    _idxs_ap = gp.lower_ap(idxs_ap)
    _out_ap = gp.lower_ap(out_ap)
    return gp.add_instruction(
        mybir.InstDMAGatherAnt(
            name=gp.bass.get_next_instruction_name(),
            ins=[*_in_ap, _idxs_ap, gp.lower_val_access(gp.to_reg(num_idxs))],
            outs=[_out_ap],
            transpose=False,
            num_idxs=num_idxs,
            elem_size=elem_size,
            stride_bytes_256=stride_256,
            gen_mode=0,
            single_packet=True,
            queue_num=queue_num,
            sbuf_tokens_per_rank=0,
            sbuf_free_dim_per_rank=0,
            sbuf_free_dim_pad_per_rank=0,
            sbuf_byte_offset=0,
        )
    )


# ---------------------------------------------------------------- device program

def _build(m):
    nc = bacc.Bacc("TRN2", target_bir_lowering=False, debug=False,
                   num_devices=NCORES, num_swdge_queues=4)
    nc._swq = 0
    H, C = m.heads, m.hid
    HC = H * C                       # 128
    NT, GW, Npad, QN = m.NT, m.GW, m.Npad, m.QN
    NPC_pad = m.NPC_pad
    R1 = HC + H                      # gathered rec1 elems: h(128)+s(4)
    R2 = C + 1                       # gathered rec2 elems: h2(32)+s2(1)

    def ein(name, shape, dt):
        return nc.dram_tensor(name, shape, dt, kind="ExternalInput")

    xT_in = ein("xT_in", [P, Npad], f16)
    W1e = ein("W1e", [HC, HC + 2 * H], f16)
    W2e = ein("W2e", [HC, C + 2], f16)
    b1_bc = ein("b1_bc", [P, HC], f32)
    b2_bc = ein("b2_bc", [P, C], f32)
    iota_bc = ein("iota_bc", [P, P], f16)
    rec_idx = ein("rec_idx", [P, sum(m.rec_cols)], i16)
    li_in = ein("li_in", [P, m.TbSum], f16)
    mask_in = ein("mask_in", [P, m.TbSum], f32)
    OT_in = ein("OT_in", [P, m.TbSum * P], f16)
    dwidx_in = ein("dwidx_in", [P, 8], i16)
    gidA_in = ein("gidA", [P, NT], f16)
    gidB_in = ein("gidB", [P, NT], f16)
    recip_in = ein("recip_in", [P, 2], f32)
    Wlin = ein("Wlin", [C, 10], f32)
    blin = ein("blin", [10, 1], f32)

    out_t = nc.dram_tensor("out", [256, 10], f32, kind="ExternalOutput")

    table1 = nc.dram_tensor("table1", [Npad, 256], f16, kind="Internal")
    d1glob = nc.dram_tensor("d1glob", [GW, 4 * P], f16, kind="Internal")
    h1xT = nc.dram_tensor("h1xT", [P, NPC_pad], f16, kind="Internal")
    agT = nc.dram_tensor("agT", [NCORES * P, NPC_pad], f16, kind="Internal",
                         addr_space="Shared")
    table2 = nc.dram_tensor("table2", [Npad, P], f16, kind="Internal")
    d2glob = nc.dram_tensor("d2glob", [GW, P], f16, kind="Internal")
    hfin = nc.dram_tensor("hfin", [NPC_pad, C], f16, kind="Internal")
    po_in = nc.dram_tensor("po_in", [256, C], f32, kind="Internal")
    po_out = nc.dram_tensor("po_out", [256, C], f32, kind="Internal",
                            addr_space="Shared")

    AL = mybir.AluOpType
    rg = [list(range(NCORES))]

    with tile.TileContext(nc) as tc:
        _phaseW(nc, tc, m, src="x", xsrc=xT_in, We=W1e, wcols=HC + 2 * H,
                scols=H, table=table1, tcols=256, dglob=d1glob)
        _gat_layer(nc, tc, m, layer=1, table=table1, dglob=d1glob,
                   rec_elem=R1, tstep=256, nh=H, ch=C, b_bc=b1_bc,
                   iota_bc=iota_bc, rec_idx=rec_idx, li_in=li_in,
                   mask_in=mask_in, OT_in=OT_in, dwidx_in=dwidx_in,
                   out_norm=h1xT, out_is_T=True)
        nc.gpsimd.collective_compute(
            kind="AllGather", op=AL.bypass, replica_groups=rg,
            ins=[h1xT[:, :]], outs=[agT[:, :]])
        _phaseW(nc, tc, m, src="ag", xsrc=agT, We=W2e, wcols=C + 2,
                scols=1, table=table2, tcols=P, dglob=d2glob)
        _gat_layer(nc, tc, m, layer=2, table=table2, dglob=d2glob,
                   rec_elem=R2, tstep=P, nh=1, ch=C, b_bc=b2_bc,
                   iota_bc=iota_bc, rec_idx=rec_idx, li_in=li_in,
                   mask_in=mask_in, OT_in=OT_in, dwidx_in=dwidx_in,
                   out_norm=hfin, out_is_T=False)
        _pool_final(nc, tc, m, hfin, gidA_in, gidB_in, iota_bc, recip_in,
                    Wlin, blin, po_in, po_out, out_t, rg)

    nc.compile()
    return nc


def _phaseW(nc, tc, m, src, xsrc, We, wcols, scols, table, tcols, dglob):
    """Replicated: rec rows [h | s | d] for ALL nodes + transposed-d window rows.

    src=='x': lhsT tiles from xT_in [128, Npad] (global window i cols).
    src=='ag': lhsT tiles from agT [(W%8)*128 block rows, (W//8)*128 cols].
    """
    AF = mybir.ActivationFunctionType
    GW = m.GW
    hcols = wcols - 2 * scols        # payload h cols
    reccols = hcols + scols          # cols copied to table rows (h + s)
    with tc.tile_pool(name=f"pw{src}", bufs=3) as sb, \
         tc.tile_pool(name=f"pw{src}c", bufs=1) as sbc, \
         tc.tile_pool(name=f"pw{src}d", bufs=2) as sd, \
         tc.tile_pool(name=f"pw{src}ps", bufs=3, space="PSUM") as ps:
        Wt = sbc.tile([P, wcols], f16)
        nc.sync.dma_start(out=Wt[:], in_=We[:, :])
        ident = sbc.tile([P, P], f16)
        make_identity(nc, ident[:])
        # persistent rec buffers so the junk tail stays initialized
        rec_bufs = [sbc.tile([P, tcols], f16, name=f"recb{k}_{src}")
                    for k in range(3)]
        for rb in rec_bufs:
            if tcols > wcols:
                nc.gpsimd.memset(rb[:, wcols:], 0.0)
        DS = 8                       # windows per d-stage flush
        dstage = None
        for i in range(GW):
            xt = sb.tile([P, P], f16, tag="xt")
            if src == "x":
                nc.sync.dma_start(out=xt[:], in_=xsrc[:, i * P:(i + 1) * P])
            else:
                cW, wW = i % NCORES, i // NCORES
                nc.sync.dma_start(
                    out=xt[:],
                    in_=xsrc[cW * P:(cW + 1) * P, wW * P:(wW + 1) * P])
            psA = ps.tile([P, wcols], f32, tag="psA")
            nc.tensor.matmul(out=psA[:], lhsT=xt[:], rhs=Wt[:],
                             start=True, stop=True)
            rec = rec_bufs[i % 3]
            # copy h+s+d (wcols) in one op; table row reads only h+s
            if i % 2 == 0:
                nc.scalar.activation(out=rec[:, 0:wcols], in_=psA[:],
                                     func=AF.Copy)
            else:
                nc.vector.tensor_copy(out=rec[:, 0:wcols], in_=psA[:])
            nc.sync.dma_start(out=table[i * P:(i + 1) * P, :], in_=rec[:])
            # transposed d row for this window
            if i % DS == 0:
                dstage = sd.tile([scols, DS, P], f16, tag="dstage")
            psT = ps.tile([scols, P], f16, tag="psT")
            nc.tensor.transpose(out=psT[:], in_=rec[:, hcols + scols:wcols],
                                identity=ident[:])
            nc.vector.tensor_copy(out=dstage[:, i % DS, :], in_=psT[:])
            if i % DS == DS - 1 or i == GW - 1:
                i0 = (i // DS) * DS
                nk = i - i0 + 1
                nc.sync.dma_start(
                    out=dglob[i0:i0 + nk, :].rearrange(
                        "r (p c) -> p r c", p=scols),
                    in_=dstage[:, 0:nk, :])


def _gat_layer(nc, tc, m, layer, table, dglob, rec_elem, tstep, nh, ch, b_bc,
               iota_bc, rec_idx, li_in, mask_in, OT_in, dwidx_in,
               out_norm, out_is_T):
    AL = mybir.AluOpType
    AF = mybir.ActivationFunctionType
    hcols = nh * ch                  # 128 or 32
    rcols = nh * (ch + 1)            # weighted-rec cols per tile (132 / 33)
    rec_col_off = np.cumsum([0] + m.rec_cols)
    li_col_off = np.cumsum([0] + m.li_cols)

    with tc.tile_pool(name=f"L{layer}", bufs=2) as sb, \
         tc.tile_pool(name=f"L{layer}c", bufs=1) as sbc, \
         tc.tile_pool(name=f"L{layer}e", bufs=3) as se, \
         tc.tile_pool(name=f"L{layer}ps", bufs=2, space="PSUM") as ps, \
         tc.tile_pool(name=f"L{layer}pw", bufs=3, space="PSUM") as pws:
        iota = sbc.tile([P, P], f16)
        nc.sync.dma_start(out=iota[:], in_=iota_bc[:, :])
        bt = sbc.tile([P, hcols], f32)
        nc.sync.dma_start(out=bt[:], in_=b_bc[:, 0:hcols])
        ident = sbc.tile([P, P], f16)
        make_identity(nc, ident[:])
        dwx = sbc.tile([P, 8], i16)
        nc.sync.dma_start(out=dwx[:], in_=dwidx_in[:, :])
        # all windows' transposed d: [t, h, w]
        dall = sbc.tile([P, nh, P], f16)
        nc.gpsimd.dma_gather(
            out_ap=dall[:], in_ap=dglob[0:m.GW, 0:nh * P],
            idxs_ap=dwx[:], num_idxs=P, num_idxs_reg=P,
            elem_size=nh * P, elem_step=None, transpose=True,
            queue_num=nc._swq % 4)
        nc._swq += 1
        for b in range(m.NB):
            B = m.batches[b]
            Tb = B["Tb"]
            if Tb == 0:
                continue
            lc0 = li_col_off[b]
            li = sb.tile([P, Tb], f16, tag="li")
            nc.sync.dma_start(out=li[:], in_=li_in[:, lc0:lc0 + Tb])
            msk = sb.tile([P, Tb], f32, tag="msk")
            nc.sync.dma_start(out=msk[:], in_=mask_in[:, lc0:lc0 + Tb])
            OT = sb.tile([P, Tb, P], f16, tag="OT")
            nc.sync.dma_start(
                out=OT[:],
                in_=OT_in[:, lc0 * P:(lc0 + Tb) * P].rearrange(
                    "p (t c) -> p t c", t=Tb))
            # ---- gathers (rec rows per quarter region)
            rec = sb.tile([P, Tb, rec_elem], f16, tag="rec")
            for q in range(4):
                Rq = B["Rq"][q]
                if Rq == 0:
                    continue
                ci = rec_col_off[4 * b + q]
                cn = m.rec_cols[4 * b + q]
                rxt = sb.tile([P, cn], i16, tag=f"rxt{q}")
                nc.sync.dma_start(out=rxt[:], in_=rec_idx[:, ci:ci + cn])
                r0 = B["reg_base"][q]
                for c0 in range(0, Rq, CH):
                    cT = min(CH, Rq - c0)
                    qn = nc._swq % 4
                    nc._swq += 1
                    _dma_gather_raw(
                        nc.gpsimd,
                        out_ap=rec[:, r0 + c0:r0 + c0 + cT, :],
                        in_ap=table[q * m.QN:(q + 1) * m.QN, 0:rec_elem],
                        idxs_ap=rxt[:, c0 * 8:(c0 + cT) * 8],
                        num_idxs=cT * P, elem_size=rec_elem, elem_step=tstep,
                        queue_num=qn)
            # ---- per-edge d via PE: psD[s, (j,h)] = OT_j^T @ dwin
            psD = ps.tile([P, Tb, nh], f32, tag="psD")
            for w in B["ws"]:
                dw = sb.tile([P, nh], f16, tag=f"dw{w % BWIN}")
                nc.vector.tensor_copy(out=dw[:], in_=dall[:, :, w])
                for q in range(4):
                    Tq = int(m.Twq[w, q])
                    for j in range(B["blk"][(w, q)], B["blk"][(w, q)] + Tq):
                        nc.tensor.matmul(out=psD[:, j, :], lhsT=OT[:, j, :],
                                         rhs=dw[:], start=True, stop=True)
            # ---- bulk attention weights w = exp(lrelu(s + d) + mask)
            dsb = sb.tile([P, Tb, nh], f16, tag="dsb")
            nc.vector.tensor_copy(out=dsb[:], in_=psD[:])
            t4 = sb.tile([P, Tb, nh], f32, tag="t4")
            nc.vector.tensor_tensor(out=t4[:], in0=dsb[:],
                                    in1=rec[:, :, hcols:hcols + nh], op=AL.add)
            nc.vector.tensor_tensor(
                out=t4[:], in0=t4[:],
                in1=msk[:].unsqueeze(2).to_broadcast([P, Tb, nh]), op=AL.add)
            t5 = sb.tile([P, Tb, nh], f32, tag="t5")
            nc.vector.tensor_scalar_mul(t5[:], t4[:], NEG_SLOPE)
            nc.vector.tensor_tensor(out=t4[:], in0=t4[:], in1=t5[:], op=AL.max)
            w4 = sb.tile([P, Tb, nh], f32, tag="w4")
            nc.scalar.activation(out=w4[:], in_=t4[:], func=AF.Exp)
            w4h = sb.tile([P, Tb, nh], f16, tag="w4h")
            nc.vector.tensor_copy(out=w4h[:], in_=w4[:])
            # ---- bulk weighted records [w*h | w] per head
            wrec = sb.tile([P, Tb, nh, ch + 1], f16, tag="wrec")
            nc.vector.tensor_tensor(
                out=wrec[:, :, :, 0:ch],
                in0=rec[:, :, 0:hcols].rearrange("p t (h c) -> p t h c", h=nh),
                in1=w4h[:].unsqueeze(3).to_broadcast([P, Tb, nh, ch]),
                op=AL.mult)
            nc.scalar.copy(out=wrec[:, :, :, ch], in_=w4h[:])
            # ---- one-hots for the whole batch
            O = sb.tile([P, Tb, P], f16, tag="O")
            nc.vector.tensor_tensor(
                out=O[:],
                in0=iota[:].unsqueeze(1).to_broadcast([P, Tb, P]),
                in1=li[:].unsqueeze(2).to_broadcast([P, Tb, P]),
                op=AL.is_equal)
            # ---- aggregation + window epilogue
            for w in B["ws"]:
                nw = int(m.Twq[w, :].sum())
                if nw == 0:
                    continue
                pw = pws.tile([P, rcols], f32, tag="pw")
                seen = 0
                for q in range(4):
                    Tq = int(m.Twq[w, q])
                    for j in range(B["blk"][(w, q)], B["blk"][(w, q)] + Tq):
                        nc.tensor.matmul(
                            out=pw[:], lhsT=O[:, j, :],
                            rhs=wrec[:, j, :, :].rearrange("p h c -> p (h c)"),
                            start=(seen == 0), stop=(seen == nw - 1))
                        seen += 1
                pwv = pw[:].rearrange("p (h c) -> p h c", h=nh)
                den = se.tile([P, nh], f32, tag="den")
                nc.vector.tensor_scalar_add(den[:], pwv[:, :, ch], 1e-16)
                rcp = se.tile([P, nh], f32, tag="rcp")
                nc.vector.reciprocal(rcp[:], den[:])
                y = se.tile([P, hcols], f32, tag="y")
                nc.vector.tensor_tensor(
                    out=y[:].rearrange("p (h c) -> p h c", h=nh),
                    in0=pwv[:, :, 0:ch],
                    in1=rcp[:].unsqueeze(2).to_broadcast([P, nh, ch]),
                    op=AL.mult)
                nc.vector.tensor_tensor(out=y[:], in0=y[:], in1=bt[:], op=AL.add)
                mn = se.tile([P, hcols], f32, tag="mn")
                nc.vector.tensor_scalar_min(mn[:], y[:], 0.0)
                ex = se.tile([P, hcols], f32, tag="ex")
                nc.scalar.activation(out=ex[:], in_=mn[:], func=AF.Exp)
                nc.vector.tensor_scalar_max(y[:], y[:], 0.0)
                s2 = se.tile([P, hcols], f32, tag="s2")
                nc.vector.tensor_tensor(out=s2[:], in0=y[:], in1=ex[:], op=AL.add)
                hf = se.tile([P, hcols], f16, tag="hf")
                nc.vector.tensor_scalar_add(hf[:], s2[:], -1.0)
                if out_is_T:
                    psT = ps.tile([hcols, P], f16, tag="psT")
                    nc.tensor.transpose(out=psT[:], in_=hf[:], identity=ident[:])
                    hfT = se.tile([hcols, P], f16, tag="hfT")
                    nc.scalar.activation(out=hfT[:], in_=psT[:], func=AF.Copy)
                    nc.sync.dma_start(out=out_norm[:, w * P:(w + 1) * P],
                                      in_=hfT[:])
                else:
                    nc.sync.dma_start(out=out_norm[w * P:(w + 1) * P, :],
                                      in_=hf[:])


def _pool_final(nc, tc, m, hfin, gidA_in, gidB_in, iota_bc, recip_in,
                Wlin, blin, po_in, po_out, out_t, rg):
    AL = mybir.AluOpType
    AF = mybir.ActivationFunctionType
    C = m.hid
    with tc.tile_pool(name="pf", bufs=2) as sb, \
         tc.tile_pool(name="pfc", bufs=1) as sbc, \
         tc.tile_pool(name="pfps", bufs=1, space="PSUM") as ps:
        iota = sbc.tile([P, P], f16)
        nc.sync.dma_start(out=iota[:], in_=iota_bc[:, :])
        gA = sbc.tile([P, m.NT], f16)
        nc.sync.dma_start(out=gA[:], in_=gidA_in[:, :])
        gB = sbc.tile([P, m.NT], f16)
        nc.sync.dma_start(out=gB[:], in_=gidB_in[:, :])
        pA = ps.tile([P, C], f32, tag="pA")
        pB = ps.tile([P, C], f32, tag="pB")
        for t in range(m.NT):
            h = sb.tile([P, C], f16, tag="h")
            nc.sync.dma_start(out=h[:], in_=hfin[t * P:(t + 1) * P, :])
            for g_t, acc in ((gA, pA), (gB, pB)):
                O = sb.tile([P, P], f16, tag="Opool")
                nc.vector.tensor_tensor(
                    out=O[:], in0=iota[:],
                    in1=g_t[:, t:t + 1].to_broadcast([P, P]), op=AL.is_equal)
                nc.tensor.matmul(out=acc[:], lhsT=O[:], rhs=h[:],
                                 start=(t == 0), stop=(t == m.NT - 1))
        sA = sb.tile([P, C], f32)
        nc.vector.tensor_copy(out=sA[:], in_=pA[:])
        sB = sb.tile([P, C], f32)
        nc.vector.tensor_copy(out=sB[:], in_=pB[:])
        nc.sync.dma_start(out=po_in[0:P, :], in_=sA[:])
        nc.sync.dma_start(out=po_in[P:256, :], in_=sB[:])
        nc.gpsimd.collective_compute(
            kind="AllReduce", op=AL.add, replica_groups=rg,
            ins=[po_in[:, :]], outs=[po_out[:, :]])
        rcp = sbc.tile([P, 2], f32)
        nc.sync.dma_start(out=rcp[:], in_=recip_in[:, :])
        ident = sbc.tile([P, P], f32)
        make_identity(nc, ident[:])
        WT = sbc.tile([C, 10], f32)
        nc.sync.dma_start(out=WT[:], in_=Wlin[:, :])
        bl = sbc.tile([10, 1], f32)
        nc.sync.dma_start(out=bl[:], in_=blin[:, :])
        poT = sb.tile([C, 256], f32)
        for half in range(2):
            pm = sb.tile([P, C], f32, tag="pm")
            nc.sync.dma_start(out=pm[:], in_=po_out[half * P:(half + 1) * P, :])
            nc.vector.tensor_scalar(
                out=pm[:], in0=pm[:], scalar1=rcp[:, half:half + 1],
                scalar2=None, op0=AL.mult)
            tp = ps.tile([C, P], f32, tag="tp")
            nc.tensor.transpose(out=tp[:], in_=pm[:], identity=ident[:])
            nc.vector.tensor_copy(out=poT[:, half * P:(half + 1) * P], in_=tp[:])
        om = ps.tile([10, 256], f32, tag="om")
        nc.tensor.matmul(out=om[:], lhsT=WT[:], rhs=poT[:], start=True, stop=True)
        ob = sb.tile([10, 256], f32)
        nc.scalar.activation(out=ob[:], in_=om[:], func=AF.Identity, bias=bl[:, 0:1])
        for half in range(2):
            tp2 = ps.tile([P, 10], f32, tag="tp2")
            nc.tensor.transpose(out=tp2[:], in_=ob[:, half * P:(half + 1) * P],
                                identity=ident[0:10, 0:10])
            oo = sb.tile([P, 10], f32, tag="oo")
            nc.vector.tensor_copy(out=oo[:], in_=tp2[:])
            nc.sync.dma_start(out=out_t[half * P:(half + 1) * P, :], in_=oo[:])


# ---------------------------------------------------------------- entry point

def kernel(x, edge_index, batch, W1, a_src1, a_dst1, b1, W2, a_src2, a_dst2,
           b2, W_lin, b_lin):
    global _last_exec_ns
    x = np.asarray(x)
    N, IN_C = x.shape
    heads, hid = np.asarray(a_src1).shape
    m = _host_prep(x, np.asarray(edge_index), np.asarray(batch), heads, hid)

    nc = _build(m)

    h16 = np.float16
    H, C = heads, hid
    HC = H * C

    # extended weights: [W | W@a_src per head | W@a_dst per head]
    W1f = np.asarray(W1, np.float64)
    a_s1 = np.asarray(a_src1, np.float64)
    a_d1 = np.asarray(a_dst1, np.float64)
    ws1 = np.stack([W1f[:, h * C:(h + 1) * C] @ a_s1[h] for h in range(H)], 1)
    wd1 = np.stack([W1f[:, h * C:(h + 1) * C] @ a_d1[h] for h in range(H)], 1)
    W1e = np.concatenate([W1f, ws1, wd1], 1).astype(h16)          # [128, 136]
    W2f = np.asarray(W2, np.float64)
    ws2 = W2f @ np.asarray(a_src2, np.float64)[0]
    wd2 = W2f @ np.asarray(a_dst2, np.float64)[0]
    W2e = np.concatenate([W2f, ws2[:, None], wd2[:, None]], 1).astype(h16)

    # permuted transposed x [128, Npad]
    xTp = np.zeros((IN_C, m.Npad), h16)
    xTp[:, m.node2slot] = np.asarray(x).T.astype(h16)

    iota = np.tile(np.arange(P, dtype=np.float64), (P, 1)).astype(h16)
    recip2 = np.stack([m.recip[0:P], m.recip[P:256]], 1).astype(np.float32)

    in_maps = []
    for c in range(NCORES):
        pc = m.per_core[c]
        in_maps.append({
            "xT_in": xTp,
            "W1e": W1e,
            "W2e": W2e,
            "b1_bc": np.tile(np.asarray(b1).reshape(1, -1), (P, 1)).astype(np.float32),
            "b2_bc": np.tile(np.asarray(b2).reshape(1, -1), (P, 1)).astype(np.float32),
            "iota_bc": iota,
            "rec_idx": pc["rec_idx"],
            "li_in": pc["li"],
            "mask_in": pc["mask"],
            "OT_in": pc["OT"],
            "dwidx_in": pc["dwidx"],
            "gidA": pc["gidA"],
            "gidB": pc["gidB"],
            "recip_in": recip2,
            "Wlin": np.asarray(W_lin).astype(np.float32),
            "blin": np.asarray(b_lin).reshape(10, 1).astype(np.float32),
        })

    import os
    if os.environ.get("GAT_SIM"):
        from concourse.bass_interp import MultiCoreSim
        mcs = MultiCoreSim(nc, NCORES, require_finite=False, require_nnan=False)
        for c in range(NCORES):
            core = mcs.cores[c]
            for k, v in in_maps[c].items():
                core.tensor(k)[:] = v
        mcs.simulate()
        return np.ascontiguousarray(np.asarray(mcs.cores[0].mem_tensor("out")))

    want_trace = bool(os.environ.get("GAT_TRACE"))
    if want_trace:
        _install_ntff_hook()
    try:
        res = run_bass_kernel_spmd(nc, in_maps, core_ids=list(range(NCORES)),
                                   trace=want_trace)
    except ModuleNotFoundError:
        res = run_bass_kernel_spmd(nc, in_maps, core_ids=list(range(NCORES)),
                                   trace=False)
    _last_exec_ns = res.exec_time_ns
    return np.ascontiguousarray(res.results[0]["out"])


def run(x, edge_index, batch, W1, a_src1, a_dst1, b1, W2, a_src2, a_dst2,
        b2, W_lin, b_lin):
    return kernel(x, edge_index, batch, W1, a_src1, a_dst1, b1, W2, a_src2,
                  a_dst2, b2, W_lin, b_lin)


# revision 13
# speedup vs baseline: 1.6600x; 1.6600x over previous
"""GAT (2-layer, 4/1 heads) on 8 trn2 NeuronCores via Bass/Tile.

Strategy (v2, dst-partitioned edge-major with host-built transposed one-hots):
- Node permutation: nodes dealt degree-sorted (snake) into 8*NT windows of 128
  dsts so every window has near-equal edge count (minimal tile padding).
- Phase A (replicated): every core computes the FULL rec table
  [h | s | d] = xT-tile @ [W1 | W1@a_src | W1@a_dst] for all nodes (no
  collective), writing 512B-stride rows + a per-window transposed d table.
- GAT layer: edges (+self loops) grouped per (window, src-quarter), 128/tile.
  Per tile: per-edge d via PE matmul with a host-uploaded transposed one-hot
  (OT); bulk exp(lrelu(s+d))+mask per batch; bulk per-head weighting of the
  gathered records; aggregation via (DVE-built one-hot) @ weighted-rec matmul
  accumulating [dst,132] in PSUM; per-window softmax-normalize + bias + ELU.
- Records fetched with InstDMAGatherAnt (int16 idx, quarter-relative).
- Layer-1 output stored transposed [128f, NPC]; one AllGather; Phase C
  (replicated) computes the layer-2 rec table for all nodes.
- Final: graph mean-pool via one-hot matmuls, AllReduce, tiny linear.
"""

import math

import numpy as np
import ml_dtypes

import concourse.bass as bass
import concourse.mybir as mybir
import concourse.tile as tile
from concourse import bacc
from concourse.bass_utils import run_bass_kernel_spmd
from concourse.masks import make_identity

NCORES = 8
P = 128
NEG_SLOPE = 0.2
BWIN = 3          # windows per batch
CH = 8            # tiles per gather chunk (1024-index HW limit)

f16 = mybir.dt.float16
f32 = mybir.dt.float32
i16 = mybir.dt.int16

_last_exec_ns = None


def _install_ntff_hook():
    """Provide antenv.axon_hooks (missing on this image) so trace=True works."""
    import sys
    import types
    try:
        from antenv import axon_hooks  # noqa: F401
        return
    except ImportError:
        pass
    import antenv
    mod = types.ModuleType("antenv.axon_hooks")
    mod._hook = None
    mod.set_axon_ntff_profile_hook = lambda h: setattr(mod, "_hook", h)
    mod.get_axon_ntff_profile_hook = lambda: mod._hook
    sys.modules["antenv.axon_hooks"] = mod
    antenv.axon_hooks = mod
    try:
        from trn_agent_boot.trn_boot import _ntff_profile_via_ctypes
        mod._hook = _ntff_profile_via_ctypes("/opt/axon/libaxon_pjrt.so")
    except Exception:
        mod._hook = None
    import concourse.bass_utils as bu
    bu.upload_artifacts = lambda tmpdir: f"local:{tmpdir}"


# ---------------------------------------------------------------- host helpers

def _wrap16(flat, pad_val=0):
    """int16 index list -> [128, ceil(n/16)] wrapped+replicated layout."""
    n = len(flat)
    cols = (n + 15) // 16
    a = np.full(cols * 16, pad_val, np.int16)
    a[:n] = flat
    w = a.reshape(cols, 16).T  # [16, cols]
    return np.tile(w, (8, 1))  # [128, cols]


def _slotmajor(flat, T, dtype):
    """slot-stream [T*128] -> [128, T] (slot i -> partition i%128, tile i//128)."""
    return np.ascontiguousarray(flat.reshape(T, P).T.astype(dtype))


class Meta:
    pass


def _host_prep(x, edge_index, batch, heads, hid):
    N = x.shape[0]
    NT = (N + NCORES * P - 1) // (NCORES * P)   # windows per core
    GW = NCORES * NT                             # global windows
    Npad = GW * P
    assert GW % 4 == 0
    QN = Npad // 4
    assert QN < 32768

    E0 = edge_index.shape[1]
    src = np.concatenate([np.asarray(edge_index[0]), np.arange(N)]).astype(np.int64)
    dst = np.concatenate([np.asarray(edge_index[1]), np.arange(N)]).astype(np.int64)

    # degree-balanced snake deal of nodes into GW windows of 128 slots
    deg = np.bincount(dst, minlength=N)
    order = np.argsort(-deg, kind="stable")
    node2slot = np.empty(N, np.int64)
    idx = np.arange(N)
    r = idx // GW                                 # round = slot-within-window
    pos = idx % GW
    W = np.where(r % 2 == 0, pos, GW - 1 - pos)   # snake
    node2slot[order] = W * P + r
    # global window W -> (core W%8, local window W//8)

    sslot = node2slot[src]
    dslot = node2slot[dst]
    Wd = dslot // P
    core_of = Wd % NCORES
    w_of = Wd // NCORES
    t_of = dslot % P
    q_of = sslot // QN

    # per (core, local window, quarter) edge lists
    cell = [[[None] * 4 for _ in range(NT)] for _ in range(NCORES)]
    for c in range(NCORES):
        mc = core_of == c
        sc, wc, tc, qc = sslot[mc], w_of[mc], t_of[mc], q_of[mc]
        for w in range(NT):
            mw = wc == w
            sw, tw, qw = sc[mw], tc[mw], qc[mw]
            for q in range(4):
                mq = qw == q
                cell[c][w][q] = (sw[mq], tw[mq])

    # rebalance: int16 idx reaches SLACK rows past each quarter, so edges
    # near a quarter's start may be processed by the previous block. Pack
    # blocks 3,2,1 down to coordinated multiples of 128 (q0 absorbs slack).
    SLACK = 32768 - QN
    for w in range(NT):
        for q in (3, 2, 1):
            ns = [len(cell[c][w][q][0]) for c in range(NCORES)]
            els = [int((cell[c][w][q][0] < q * QN + SLACK).sum())
                   for c in range(NCORES)]
            B = 128 * max(-((ns[c] - els[c]) // -128) for c in range(NCORES))
            for c in range(NCORES):
                k = max(0, ns[c] - B)
                if k == 0:
                    continue
                sq, tq = cell[c][w][q]
                mv = np.nonzero(sq < q * QN + SLACK)[0][:k]
                keep = np.ones(len(sq), bool)
                keep[mv] = False
                sp, tp = cell[c][w][q - 1]
                cell[c][w][q - 1] = (np.concatenate([sp, sq[mv]]),
                                     np.concatenate([tp, tq[mv]]))
                cell[c][w][q] = (sq[keep], tq[keep])
    m_SLACK = SLACK

    # equalized tile counts per (window, quarter)
    Twq = np.zeros((NT, 4), np.int64)
    for w in range(NT):
        for q in range(4):
            mx = max(len(cell[c][w][q][0]) for c in range(NCORES))
            Twq[w, q] = (mx + P - 1) // P

    m = Meta()
    m.N, m.NT, m.GW, m.Npad, m.QN = N, NT, GW, Npad, QN
    m.SLACK = m_SLACK
    m.NPC_pad = NT * P
    m.heads, m.hid = heads, hid
    m.Twq = Twq
    m.node2slot = node2slot

    NB = (NT + BWIN - 1) // BWIN
    m.NB = NB
    m.batches = []
    for b in range(NB):
        ws = list(range(b * BWIN, min((b + 1) * BWIN, NT)))
        Rq = [int(Twq[ws, q].sum()) for q in range(4)]
        Tb = sum(Rq)
        reg_base = np.cumsum([0] + Rq)[:4]
        blk = {}
        for q in range(4):
            off = reg_base[q]
            for w in ws:
                blk[(w, q)] = int(off)
                off += int(Twq[w, q])
        m.batches.append(dict(ws=ws, Rq=Rq, Tb=Tb, blk=blk, reg_base=reg_base))

    m.rec_cols = []   # idx col counts per (b,q)
    m.li_cols = []    # li/mask col counts per b (=Tb)
    TbSum = sum(B["Tb"] for B in m.batches)
    m.TbSum = TbSum

    I128 = np.eye(P, dtype=np.float16)
    per_core = []
    for c in range(NCORES):
        rec_idx_cols = []
        li_all = np.zeros(TbSum * P, np.int64)
        mask_all = np.zeros(TbSum * P, np.float32)
        pos0 = 0
        for b in range(NB):
            B = m.batches[b]
            for q in range(4):
                r_flat = np.zeros(B["Rq"][q] * P, np.int64)
                for w in B["ws"]:
                    sw, tw = cell[c][w][q]
                    t0 = B["blk"][(w, q)] - B["reg_base"][q]
                    nsl = int(Twq[w, q]) * P
                    k = len(sw)
                    rr = np.zeros(nsl, np.int64)
                    ll = np.zeros(nsl, np.int64)
                    mm_ = np.full(nsl, -30000.0, np.float32)
                    rr[:k] = sw - q * QN
                    ll[:k] = tw
                    mm_[:k] = 0.0
                    r_flat[t0 * P:t0 * P + nsl] = rr
                    g0 = pos0 + (B["reg_base"][q] + t0) * P
                    li_all[g0:g0 + nsl] = ll
                    mask_all[g0:g0 + nsl] = mm_
                rec_idx_cols.append(_wrap16(r_flat.astype(np.int16)))
                if c == 0:
                    m.rec_cols.append(rec_idx_cols[-1].shape[1])
            if c == 0:
                m.li_cols.append(B["Tb"])
            pos0 += B["Tb"] * P
        li_sm = _slotmajor(li_all, TbSum, np.float16)
        mask_sm = _slotmajor(mask_all, TbSum, np.float32)
        OT = np.ascontiguousarray(
            I128[:, li_all.astype(np.int64).reshape(TbSum, P)]
            .transpose(0, 1, 2).reshape(P, TbSum * P))
        # OT[t, j*128+s] = 1 iff li(slot s of tile j) == t ; li_all is
        # slot-stream (tile-major), so cols are already (j, s)-ordered.
        dwidx = np.zeros(P, np.int64)
        dwidx[:NT] = np.arange(NT) * NCORES + c    # global W of local w
        per_core.append(dict(
            rec_idx=np.concatenate(rec_idx_cols, 1),
            li=li_sm, mask=mask_sm, OT=OT.astype(np.float16),
            dwidx=_wrap16(dwidx.astype(np.int16)),
        ))

    # graph pooling metadata (slot order)
    batch = np.asarray(batch).astype(np.int64)
    G = int(np.max(batch)) + 1
    m.G = G
    assert G <= 256
    counts = np.bincount(batch, minlength=256)
    m.recip = (1.0 / np.maximum(counts, 1)).astype(np.float32)
    for c in range(NCORES):
        gid = np.full(m.NPC_pad, -1, np.int64)
        # local row w*128+t <-> global slot (w*8+c)*128+t
        slots = ((np.arange(m.NPC_pad) // P) * NCORES + c) * P + (np.arange(m.NPC_pad) % P)
        node_of_slot = np.full(Npad, -1, np.int64)
        node_of_slot[node2slot] = np.arange(N)
        nds = node_of_slot[slots]
        real = nds >= 0
        gid[real] = batch[nds[real]]
        gA = gid.astype(np.float64)
        gB = np.where(gid >= 0, gid - 128, -1).astype(np.float64)
        per_core[c]["gidA"] = _slotmajor(gA, NT, np.float16)
        per_core[c]["gidB"] = _slotmajor(gB, NT, np.float16)
    m.per_core = per_core
    return m


# ---------------------------------------------------------------- raw dma_gather

def _dma_gather_raw(gp, out_ap, in_ap, idxs_ap, num_idxs, elem_size, elem_step,
                    queue_num=0):
    """dma_gather without the elem%256B assert (stride must be 256B-mult)."""
    from concourse import ap_utils
    from concourse._compat import exact_div
    assert idxs_ap.dtype == i16
    assert in_ap.dtype == out_ap.dtype
    assert ap_utils.ap_is_contiguous(in_ap.ap[1:])
    assert ap_utils.ap_is_contiguous(out_ap.ap[1:])
    assert ap_utils.ap_is_contiguous(idxs_ap.ap[1:])
    assert in_ap.ap[0][0] == elem_step
    stride_bytes = elem_step * mybir.dt.size(in_ap.dtype)
    stride_256 = exact_div(stride_bytes, 256)
    assert stride_256 < 256
    _in_ap = gp.lower_ap_dma(in_ap, for_custom_bir_dma=True)
    _idxs_ap = gp.lower_ap(idxs_ap)
    _out_ap = gp.lower_ap(out_ap)
    return gp.add_instruction(
        mybir.InstDMAGatherAnt(
            name=gp.bass.get_next_instruction_name(),
            ins=[*_in_ap, _idxs_ap, gp.lower_val_access(gp.to_reg(num_idxs))],
            outs=[_out_ap],
            transpose=False,
            num_idxs=num_idxs,
            elem_size=elem_size,
            stride_bytes_256=stride_256,
            gen_mode=0,
            single_packet=True,
            queue_num=queue_num,
            sbuf_tokens_per_rank=0,
            sbuf_free_dim_per_rank=0,
            sbuf_free_dim_pad_per_rank=0,
            sbuf_byte_offset=0,
        )
    )


# ---------------------------------------------------------------- device program

def _build(m):
    nc = bacc.Bacc("TRN2", target_bir_lowering=False, debug=False,
                   num_devices=NCORES, num_swdge_queues=4)
    nc._swq = 0
    H, C = m.heads, m.hid
    HC = H * C                       # 128
    NT, GW, Npad, QN = m.NT, m.GW, m.Npad, m.QN
    NPC_pad = m.NPC_pad
    R1 = HC + H                      # gathered rec1 elems: h(128)+s(4)
    R2 = C + 1                       # gathered rec2 elems: h2(32)+s2(1)

    def ein(name, shape, dt):
        return nc.dram_tensor(name, shape, dt, kind="ExternalInput")

    xT_in = ein("xT_in", [P, Npad], f16)
    W1e = ein("W1e", [HC, HC + 2 * H], f16)
    W2e = ein("W2e", [HC, C + 2], f16)
    b1_bc = ein("b1_bc", [P, HC], f32)
    b2_bc = ein("b2_bc", [P, C], f32)
    iota_bc = ein("iota_bc", [P, P], f16)
    Tbmax = max(B["Tb"] for B in m.batches)
    iotaRep_in = ein("iotaRep_in", [P, Tbmax * P], f16)
    rec_idx = ein("rec_idx", [P, sum(m.rec_cols)], i16)
    li_in = ein("li_in", [P, m.TbSum], f16)
    mask_in = ein("mask_in", [P, m.TbSum], f32)
    OT_in = ein("OT_in", [P, m.TbSum * P], f16)
    dwidx_in = ein("dwidx_in", [P, 8], i16)
    gidA_in = ein("gidA", [P, NT], f16)
    gidB_in = ein("gidB", [P, NT], f16)
    recip_in = ein("recip_in", [P, 2], f32)
    Wlin = ein("Wlin", [C, 10], f32)
    blin = ein("blin", [10, 1], f32)

    out_t = nc.dram_tensor("out", [256, 10], f32, kind="ExternalOutput")

    table1 = nc.dram_tensor("table1", [Npad, 256], f16, kind="Internal")
    d1glob = nc.dram_tensor("d1glob", [GW, 4 * P], f16, kind="Internal")
    h1xT = nc.dram_tensor("h1xT", [P, NPC_pad], f16, kind="Internal")
    agT = nc.dram_tensor("agT", [NCORES * P, NPC_pad], f16, kind="Internal",
                         addr_space="Shared")
    table2 = nc.dram_tensor("table2", [Npad, P], f16, kind="Internal")
    d2glob = nc.dram_tensor("d2glob", [GW, P], f16, kind="Internal")
    hfin = nc.dram_tensor("hfin", [NPC_pad, C], f16, kind="Internal")
    po_in = nc.dram_tensor("po_in", [256, C], f32, kind="Internal")
    po_out = nc.dram_tensor("po_out", [256, C], f32, kind="Internal",
                            addr_space="Shared")

    AL = mybir.AluOpType
    rg = [list(range(NCORES))]

    with tile.TileContext(nc) as tc:
        _phaseW(nc, tc, m, src="x", xsrc=xT_in, We=W1e, wcols=HC + 2 * H,
                scols=H, table=table1, tcols=256, dglob=d1glob)
        _gat_layer(nc, tc, m, layer=1, table=table1, dglob=d1glob,
                   rec_elem=R1, tstep=256, nh=H, ch=C, b_bc=b1_bc,
                   iota_bc=iota_bc, iotaRep_in=iotaRep_in, rec_idx=rec_idx,
                   li_in=li_in, mask_in=mask_in, OT_in=OT_in,
                   dwidx_in=dwidx_in, out_norm=h1xT, out_is_T=True)
        nc.gpsimd.collective_compute(
            kind="AllGather", op=AL.bypass, replica_groups=rg,
            ins=[h1xT[:, :]], outs=[agT[:, :]])
        _phaseW(nc, tc, m, src="ag", xsrc=agT, We=W2e, wcols=C + 2,
                scols=1, table=table2, tcols=P, dglob=d2glob)
        _gat_layer(nc, tc, m, layer=2, table=table2, dglob=d2glob,
                   rec_elem=R2, tstep=P, nh=1, ch=C, b_bc=b2_bc,
                   iota_bc=iota_bc, iotaRep_in=iotaRep_in, rec_idx=rec_idx,
                   li_in=li_in, mask_in=mask_in, OT_in=OT_in,
                   dwidx_in=dwidx_in, out_norm=hfin, out_is_T=False)
        _pool_final(nc, tc, m, hfin, gidA_in, gidB_in, iota_bc, recip_in,
                    Wlin, blin, po_in, po_out, out_t, rg)

    nc.compile()
    return nc


def _phaseW(nc, tc, m, src, xsrc, We, wcols, scols, table, tcols, dglob):
    """Replicated: rec rows [h | s | d] for ALL nodes + transposed-d window rows.

    Processes DS=8 window-tiles per iteration with batched DMAs to keep the
    HWDGE sequencer off the critical path.
    """
    AF = mybir.ActivationFunctionType
    GW = m.GW
    hcols = wcols - 2 * scols        # payload h cols
    DS = 8
    NG = (GW + DS - 1) // DS
    with tc.tile_pool(name=f"pw{src}", bufs=3) as sb, \
         tc.tile_pool(name=f"pw{src}c", bufs=1) as sbc, \
         tc.tile_pool(name=f"pw{src}d", bufs=2) as sd, \
         tc.tile_pool(name=f"pw{src}ps", bufs=4, space="PSUM") as ps:
        Wt = sbc.tile([P, wcols], f16)
        nc.sync.dma_start(out=Wt[:], in_=We[:, :])
        ident = sbc.tile([P, P], f16)
        make_identity(nc, ident[:])
        # persistent rec buffers so the junk tail stays initialized
        rec_bufs = [sbc.tile([P, DS, tcols], f16, name=f"recb{k}_{src}")
                    for k in range(3)]
        for rb in rec_bufs:
            if tcols > wcols:
                nc.gpsimd.memset(rb[:, :, wcols:], 0.0)
        for g in range(NG):
            i0 = g * DS
            nk = min(DS, GW - i0)
            xb = sb.tile([P, DS, P], f16, tag="xb")
            if src == "x":
                nc.sync.dma_start(
                    out=xb[:, 0:nk, :],
                    in_=xsrc[:, i0 * P:(i0 + nk) * P].rearrange(
                        "p (k n) -> p k n", k=nk))
            else:
                nc.sync.dma_start(
                    out=xb[:, 0:nk, :],
                    in_=xsrc[0:nk * P, g * P:(g + 1) * P].rearrange(
                        "(k p) n -> p k n", p=P))
            rec = rec_bufs[g % 3]
            dstage = sd.tile([scols, DS, P], f16, tag="dstage")
            for k in range(nk):
                psA = ps.tile([P, wcols], f32, tag="psA")
                nc.tensor.matmul(out=psA[:], lhsT=xb[:, k, :], rhs=Wt[:],
                                 start=True, stop=True)
                if k % 2 == 0:
                    nc.scalar.activation(out=rec[:, k, 0:wcols], in_=psA[:],
                                         func=AF.Copy)
                else:
                    nc.vector.tensor_copy(out=rec[:, k, 0:wcols], in_=psA[:])
                psT = ps.tile([scols, P], f16, tag="psT")
                nc.tensor.transpose(out=psT[:],
                                    in_=rec[:, k, hcols + scols:wcols],
                                    identity=ident[:])
                nc.vector.tensor_copy(out=dstage[:, k, :], in_=psT[:])
            nc.sync.dma_start(
                out=table[i0 * P:(i0 + nk) * P, :].rearrange(
                    "(k p) e -> p k e", p=P),
                in_=rec[:, 0:nk, :])
            nc.sync.dma_start(
                out=dglob[i0:i0 + nk, :].rearrange("r (p c) -> p r c", p=scols),
                in_=dstage[:, 0:nk, :])


def _gat_layer(nc, tc, m, layer, table, dglob, rec_elem, tstep, nh, ch, b_bc,
               iota_bc, iotaRep_in, rec_idx, li_in, mask_in, OT_in, dwidx_in,
               out_norm, out_is_T):
    AL = mybir.AluOpType
    AF = mybir.ActivationFunctionType
    hcols = nh * ch                  # 128 or 32
    rcols = nh * (ch + 1)            # weighted-rec cols per tile (132 / 33)
    rec_col_off = np.cumsum([0] + m.rec_cols)
    li_col_off = np.cumsum([0] + m.li_cols)

    with tc.tile_pool(name=f"L{layer}", bufs=2) as sb, \
         tc.tile_pool(name=f"L{layer}c", bufs=1) as sbc, \
         tc.tile_pool(name=f"L{layer}e", bufs=3) as se, \
         tc.tile_pool(name=f"L{layer}ps", bufs=2, space="PSUM") as ps, \
         tc.tile_pool(name=f"L{layer}pw", bufs=3, space="PSUM") as pws:
        iota = sbc.tile([P, P], f16)
        nc.sync.dma_start(out=iota[:], in_=iota_bc[:, :])
        Tbmax = max(Bx["Tb"] for Bx in m.batches)
        iotaRep = sbc.tile([P, Tbmax, P], f16)
        nc.sync.dma_start(out=iotaRep[:],
                          in_=iotaRep_in[:, :].rearrange(
                              "p (t c) -> p t c", t=Tbmax))
        bt = sbc.tile([P, hcols], f32)
        nc.sync.dma_start(out=bt[:], in_=b_bc[:, 0:hcols])
        ident = sbc.tile([P, P], f16)
        make_identity(nc, ident[:])
        dwx = sbc.tile([P, 8], i16)
        nc.sync.dma_start(out=dwx[:], in_=dwidx_in[:, :])
        # all windows' transposed d: [t, h, w]
        dall = sbc.tile([P, nh, P], f16)
        nc.gpsimd.dma_gather(
            out_ap=dall[:], in_ap=dglob[0:m.GW, 0:nh * P],
            idxs_ap=dwx[:], num_idxs=P, num_idxs_reg=P,
            elem_size=nh * P, elem_step=None, transpose=True,
            queue_num=nc._swq % 4)
        nc._swq += 1
        for b in range(m.NB):
            B = m.batches[b]
            Tb = B["Tb"]
            if Tb == 0:
                continue
            lc0 = li_col_off[b]
            li = sb.tile([P, Tb], f16, tag="li")
            nc.sync.dma_start(out=li[:], in_=li_in[:, lc0:lc0 + Tb])
            msk = sb.tile([P, Tb], f32, tag="msk")
            nc.sync.dma_start(out=msk[:], in_=mask_in[:, lc0:lc0 + Tb])
            OT = sb.tile([P, Tb, P], f16, tag="OT")
            nc.sync.dma_start(
                out=OT[:],
                in_=OT_in[:, lc0 * P:(lc0 + Tb) * P].rearrange(
                    "p (t c) -> p t c", t=Tb))
            # ---- gathers (rec rows per quarter region)
            rec = sb.tile([P, Tb, rec_elem], f16, tag="rec")
            for q in range(4):
                Rq = B["Rq"][q]
                if Rq == 0:
                    continue
                ci = rec_col_off[4 * b + q]
                cn = m.rec_cols[4 * b + q]
                rxt = sb.tile([P, cn], i16, tag=f"rxt{q}")
                nc.sync.dma_start(out=rxt[:], in_=rec_idx[:, ci:ci + cn])
                r0 = B["reg_base"][q]
                lim = min(m.QN + m.SLACK, table.shape[0] - q * m.QN)
                for c0 in range(0, Rq, CH):
                    cT = min(CH, Rq - c0)
                    qn = nc._swq % 4
                    nc._swq += 1
                    _dma_gather_raw(
                        nc.gpsimd,
                        out_ap=rec[:, r0 + c0:r0 + c0 + cT, :],
                        in_ap=table[q * m.QN:q * m.QN + lim, 0:rec_elem],
                        idxs_ap=rxt[:, c0 * 8:(c0 + cT) * 8],
                        num_idxs=cT * P, elem_size=rec_elem, elem_step=tstep,
                        queue_num=qn)
            # ---- per-edge d via PE: psD[s, (j,h)] = OT_j^T @ dwin
            psD = ps.tile([P, Tb, nh], f32, tag="psD")
            for w in B["ws"]:
                dw = sb.tile([P, nh], f16, tag=f"dw{w % BWIN}")
                nc.vector.tensor_copy(out=dw[:], in_=dall[:, :, w])
                for q in range(4):
                    Tq = int(m.Twq[w, q])
                    for j in range(B["blk"][(w, q)], B["blk"][(w, q)] + Tq):
                        nc.tensor.matmul(out=psD[:, j, :], lhsT=OT[:, j, :],
                                         rhs=dw[:], start=True, stop=True)
            # ---- bulk attention weights w = exp(lrelu(s + d) + mask)
            t4 = sb.tile([P, Tb, nh], f32, tag="t4")
            nc.vector.tensor_tensor(out=t4[:], in0=psD[:],
                                    in1=rec[:, :, hcols:hcols + nh], op=AL.add)
            nc.vector.tensor_tensor(
                out=t4[:], in0=t4[:],
                in1=msk[:].unsqueeze(2).to_broadcast([P, Tb, nh]), op=AL.add)
            t5 = sb.tile([P, Tb, nh], f32, tag="t5")
            nc.vector.tensor_scalar_mul(t5[:], t4[:], NEG_SLOPE)
            nc.vector.tensor_tensor(out=t4[:], in0=t4[:], in1=t5[:], op=AL.max)
            w4h = sb.tile([P, Tb, nh], f16, tag="w4h")
            nc.scalar.activation(out=w4h[:], in_=t4[:], func=AF.Exp)
            # ---- bulk weighted records [w*h | w] per head
            wrec = sb.tile([P, Tb, nh, ch + 1], f16, tag="wrec")
            nc.vector.tensor_tensor(
                out=wrec[:, :, :, 0:ch],
                in0=rec[:, :, 0:hcols].rearrange("p t (h c) -> p t h c", h=nh),
                in1=w4h[:].unsqueeze(3).to_broadcast([P, Tb, nh, ch]),
                op=AL.mult)
            nc.scalar.copy(out=wrec[:, :, :, ch], in_=w4h[:])
            # ---- one-hots for the whole batch
            O = sb.tile([P, Tb, P], f16, tag="O")
            nc.vector.tensor_tensor(
                out=O[:],
                in0=iotaRep[:, 0:Tb, :],
                in1=li[:].unsqueeze(2).to_broadcast([P, Tb, P]),
                op=AL.is_equal)
            # ---- aggregation + window epilogue
            for w in B["ws"]:
                nw = int(m.Twq[w, :].sum())
                if nw == 0:
                    continue
                pw = pws.tile([P, rcols], f32, tag="pw")
                seen = 0
                for q in range(4):
                    Tq = int(m.Twq[w, q])
                    for j in range(B["blk"][(w, q)], B["blk"][(w, q)] + Tq):
                        nc.tensor.matmul(
                            out=pw[:], lhsT=O[:, j, :],
                            rhs=wrec[:, j, :, :].rearrange("p h c -> p (h c)"),
                            start=(seen == 0), stop=(seen == nw - 1))
                        seen += 1
                pwv = pw[:].rearrange("p (h c) -> p h c", h=nh)
                den = se.tile([P, nh], f32, tag="den")
                nc.vector.tensor_scalar_add(den[:], pwv[:, :, ch], 1e-16)
                rcp = se.tile([P, nh], f32, tag="rcp")
                nc.vector.reciprocal(rcp[:], den[:])
                y = se.tile([P, hcols], f32, tag="y")
                nc.vector.tensor_tensor(
                    out=y[:].rearrange("p (h c) -> p h c", h=nh),
                    in0=pwv[:, :, 0:ch],
                    in1=rcp[:].unsqueeze(2).to_broadcast([P, nh, ch]),
                    op=AL.mult)
                nc.vector.tensor_tensor(out=y[:], in0=y[:], in1=bt[:], op=AL.add)
                mn = se.tile([P, hcols], f32, tag="mn")
                nc.vector.tensor_scalar_min(mn[:], y[:], 0.0)
                ex = se.tile([P, hcols], f32, tag="ex")
                nc.scalar.activation(out=ex[:], in_=mn[:], func=AF.Exp)
                nc.vector.tensor_scalar_max(y[:], y[:], 0.0)
                s2 = se.tile([P, hcols], f32, tag="s2")
                nc.vector.tensor_tensor(out=s2[:], in0=y[:], in1=ex[:], op=AL.add)
                hf = se.tile([P, hcols], f16, tag="hf")
                nc.vector.tensor_scalar_add(hf[:], s2[:], -1.0)
                if out_is_T:
                    psT = ps.tile([hcols, P], f16, tag="psT")
                    nc.tensor.transpose(out=psT[:], in_=hf[:], identity=ident[:])
                    hfT = se.tile([hcols, P], f16, tag="hfT")
                    nc.scalar.activation(out=hfT[:], in_=psT[:], func=AF.Copy)
                    nc.sync.dma_start(out=out_norm[:, w * P:(w + 1) * P],
                                      in_=hfT[:])
                else:
                    nc.sync.dma_start(out=out_norm[w * P:(w + 1) * P, :],
                                      in_=hf[:])


def _pool_final(nc, tc, m, hfin, gidA_in, gidB_in, iota_bc, recip_in,
                Wlin, blin, po_in, po_out, out_t, rg):
    AL = mybir.AluOpType
    AF = mybir.ActivationFunctionType
    C = m.hid
    with tc.tile_pool(name="pf", bufs=2) as sb, \
         tc.tile_pool(name="pfc", bufs=1) as sbc, \
         tc.tile_pool(name="pfps", bufs=1, space="PSUM") as ps:
        iota = sbc.tile([P, P], f16)
        nc.sync.dma_start(out=iota[:], in_=iota_bc[:, :])
        gA = sbc.tile([P, m.NT], f16)
        nc.sync.dma_start(out=gA[:], in_=gidA_in[:, :])
        gB = sbc.tile([P, m.NT], f16)
        nc.sync.dma_start(out=gB[:], in_=gidB_in[:, :])
        pA = ps.tile([P, C], f32, tag="pA")
        pB = ps.tile([P, C], f32, tag="pB")
        for t in range(m.NT):
            h = sb.tile([P, C], f16, tag="h")
            nc.sync.dma_start(out=h[:], in_=hfin[t * P:(t + 1) * P, :])
            for g_t, acc in ((gA, pA), (gB, pB)):
                O = sb.tile([P, P], f16, tag="Opool")
                nc.vector.tensor_tensor(
                    out=O[:], in0=iota[:],
                    in1=g_t[:, t:t + 1].to_broadcast([P, P]), op=AL.is_equal)
                nc.tensor.matmul(out=acc[:], lhsT=O[:], rhs=h[:],
                                 start=(t == 0), stop=(t == m.NT - 1))
        sA = sb.tile([P, C], f32)
        nc.vector.tensor_copy(out=sA[:], in_=pA[:])
        sB = sb.tile([P, C], f32)
        nc.vector.tensor_copy(out=sB[:], in_=pB[:])
        nc.sync.dma_start(out=po_in[0:P, :], in_=sA[:])
        nc.sync.dma_start(out=po_in[P:256, :], in_=sB[:])
        nc.gpsimd.collective_compute(
            kind="AllReduce", op=AL.add, replica_groups=rg,
            ins=[po_in[:, :]], outs=[po_out[:, :]])
        rcp = sbc.tile([P, 2], f32)
        nc.sync.dma_start(out=rcp[:], in_=recip_in[:, :])
        ident = sbc.tile([P, P], f32)
        make_identity(nc, ident[:])
        WT = sbc.tile([C, 10], f32)
        nc.sync.dma_start(out=WT[:], in_=Wlin[:, :])
        bl = sbc.tile([10, 1], f32)
        nc.sync.dma_start(out=bl[:], in_=blin[:, :])
        poT = sb.tile([C, 256], f32)
        for half in range(2):
            pm = sb.tile([P, C], f32, tag="pm")
            nc.sync.dma_start(out=pm[:], in_=po_out[half * P:(half + 1) * P, :])
            nc.vector.tensor_scalar(
                out=pm[:], in0=pm[:], scalar1=rcp[:, half:half + 1],
                scalar2=None, op0=AL.mult)
            tp = ps.tile([C, P], f32, tag="tp")
            nc.tensor.transpose(out=tp[:], in_=pm[:], identity=ident[:])
            nc.vector.tensor_copy(out=poT[:, half * P:(half + 1) * P], in_=tp[:])
        om = ps.tile([10, 256], f32, tag="om")
        nc.tensor.matmul(out=om[:], lhsT=WT[:], rhs=poT[:], start=True, stop=True)
        ob = sb.tile([10, 256], f32)
        nc.scalar.activation(out=ob[:], in_=om[:], func=AF.Identity, bias=bl[:, 0:1])
        for half in range(2):
            tp2 = ps.tile([P, 10], f32, tag="tp2")
            nc.tensor.transpose(out=tp2[:], in_=ob[:, half * P:(half + 1) * P],
                                identity=ident[0:10, 0:10])
            oo = sb.tile([P, 10], f32, tag="oo")
            nc.vector.tensor_copy(out=oo[:], in_=tp2[:])
            nc.sync.dma_start(out=out_t[half * P:(half + 1) * P, :], in_=oo[:])


# ---------------------------------------------------------------- entry point

def kernel(x, edge_index, batch, W1, a_src1, a_dst1, b1, W2, a_src2, a_dst2,
           b2, W_lin, b_lin):
    global _last_exec_ns
    x = np.asarray(x)
    N, IN_C = x.shape
    heads, hid = np.asarray(a_src1).shape
    m = _host_prep(x, np.asarray(edge_index), np.asarray(batch), heads, hid)

    nc = _build(m)

    h16 = np.float16
    H, C = heads, hid
    HC = H * C

    # extended weights: [W | W@a_src per head | W@a_dst per head]
    W1f = np.asarray(W1, np.float64)
    a_s1 = np.asarray(a_src1, np.float64)
    a_d1 = np.asarray(a_dst1, np.float64)
    ws1 = np.stack([W1f[:, h * C:(h + 1) * C] @ a_s1[h] for h in range(H)], 1)
    wd1 = np.stack([W1f[:, h * C:(h + 1) * C] @ a_d1[h] for h in range(H)], 1)
    W1e = np.concatenate([W1f, ws1, wd1], 1).astype(h16)          # [128, 136]
    W2f = np.asarray(W2, np.float64)
    ws2 = W2f @ np.asarray(a_src2, np.float64)[0]
    wd2 = W2f @ np.asarray(a_dst2, np.float64)[0]
    W2e = np.concatenate([W2f, ws2[:, None], wd2[:, None]], 1).astype(h16)

    # permuted transposed x [128, Npad]
    xTp = np.zeros((IN_C, m.Npad), h16)
    xTp[:, m.node2slot] = np.asarray(x).T.astype(h16)

    iota = np.tile(np.arange(P, dtype=np.float64), (P, 1)).astype(h16)
    Tbmax = max(B["Tb"] for B in m.batches)
    iotaRep = np.tile(iota, (1, Tbmax))
    recip2 = np.stack([m.recip[0:P], m.recip[P:256]], 1).astype(np.float32)

    in_maps = []
    for c in range(NCORES):
        pc = m.per_core[c]
        in_maps.append({
            "xT_in": xTp,
            "W1e": W1e,
            "W2e": W2e,
            "b1_bc": np.tile(np.asarray(b1).reshape(1, -1), (P, 1)).astype(np.float32),
            "b2_bc": np.tile(np.asarray(b2).reshape(1, -1), (P, 1)).astype(np.float32),
            "iota_bc": iota,
            "iotaRep_in": iotaRep,
            "rec_idx": pc["rec_idx"],
            "li_in": pc["li"],
            "mask_in": pc["mask"],
            "OT_in": pc["OT"],
            "dwidx_in": pc["dwidx"],
            "gidA": pc["gidA"],
            "gidB": pc["gidB"],
            "recip_in": recip2,
            "Wlin": np.asarray(W_lin).astype(np.float32),
            "blin": np.asarray(b_lin).reshape(10, 1).astype(np.float32),
        })

    import os
    if os.environ.get("GAT_SIM"):
        from concourse.bass_interp import MultiCoreSim
        mcs = MultiCoreSim(nc, NCORES, require_finite=False, require_nnan=False)
        for c in range(NCORES):
            core = mcs.cores[c]
            for k, v in in_maps[c].items():
                core.tensor(k)[:] = v
        mcs.simulate()
        return np.ascontiguousarray(np.asarray(mcs.cores[0].mem_tensor("out")))

    want_trace = bool(os.environ.get("GAT_TRACE"))
    if want_trace:
        _install_ntff_hook()
    try:
        res = run_bass_kernel_spmd(nc, in_maps, core_ids=list(range(NCORES)),
                                   trace=want_trace)
    except ModuleNotFoundError:
        res = run_bass_kernel_spmd(nc, in_maps, core_ids=list(range(NCORES)),
                                   trace=False)
    _last_exec_ns = res.exec_time_ns
    return np.ascontiguousarray(res.results[0]["out"])


def run(x, edge_index, batch, W1, a_src1, a_dst1, b1, W2, a_src2, a_dst2,
        b2, W_lin, b_lin):
    return kernel(x, edge_index, batch, W1, a_src1, a_dst1, b1, W2, a_src2,
                  a_dst2, b2, W_lin, b_lin)
